# revision 1
# baseline (speedup 1.0000x reference)
"""Multi-head attention (B=4, S=2048, D=1024, H=16) on 8 TRN2 NeuronCores.

Sharding: 2D grid (batch x head-group). Core c = g*4 + b handles batch b and
head group g (8 heads = 512 of the 1024 embedding columns).

Per-core kernel (all matmul operands float32r: fp32-grade data processed at
1 cycle/row by the PE; every operand tile is written by a compute engine so
it is rounded to FP32r as the BIR verifier requires):
  1. x_b^T [1024, 2048] (host pre-transposes) DMA'd in chunks, rounded to
     f32r, resident in SBUF.
  2. Per head-pair p (4 pairs of 2 heads = 128 cols): Q^T/K^T/V^T
     [128, 2048] = W_pair^T @ x^T (PSUM accumulation over 8 k-chunks, bias
     added on PSUM->SBUF eviction). V^T is PE-transposed into V "normal"
     layout per head with a ones column appended, so the P@V matmul also
     emits the softmax denominator as its last output row.
  3. Attention per head in scores-transposed layout: S^T [k-tile 128,
     q 1024] = K^T_tile^T @ Q^T (head slices live at partition base 0/64,
     which the PE accepts). exp on ScalarE with scale=1/8 folded in; no
     max-subtraction (scores are ~N(0,1) by construction, exp is safe).
     attn^T[65, 512] += [V_h | 1]^T @ expS over all 16 k-tiles. Row 64 =
     sumexp -> reciprocal -> K=1 ones-matmul broadcasts it across 64
     partitions -> multiply normalizes attn^T.
  4. Partial output projection out_part [2048, 1024] = attn_c @ Wo[cols_g].
Host sums the two head-group partials per batch and adds bo.
"""
import numpy as np

B, S, D, H, DH = 4, 2048, 1024, 16, 64
NCORES = 8
GCOLS = D // 2          # 512 cols per head-group core
NPAIRS = GCOLS // 128   # 4 head-pairs per core
NKT = S // 128          # 16 k-tiles
NQT = S // 1024         # 2 q-tile-pairs of 1024
DC = D // 128           # 8 contraction chunks for projections

_COMPILED = None


def _build():
    import concourse.bass as bass
    import concourse.bacc as bacc
    import concourse.tile as tile
    from concourse import mybir
    from concourse.masks import make_identity
    from contextlib import ExitStack

    F32 = mybir.dt.float32
    F32R = mybir.dt.float32r
    EXP = mybir.ActivationFunctionType.Exp

    nc = bacc.Bacc("TRN2", target_bir_lowering=False, debug=False)
    xT = nc.dram_tensor("xT", [D, S], F32, kind="ExternalInput").ap()
    wq = nc.dram_tensor("wq", [D, GCOLS], F32, kind="ExternalInput").ap()
    wk = nc.dram_tensor("wk", [D, GCOLS], F32, kind="ExternalInput").ap()
    wv = nc.dram_tensor("wv", [D, GCOLS], F32, kind="ExternalInput").ap()
    wo = nc.dram_tensor("wo", [GCOLS, D], F32, kind="ExternalInput").ap()
    bq = nc.dram_tensor("bq", [GCOLS], F32, kind="ExternalInput").ap()
    bk = nc.dram_tensor("bk", [GCOLS], F32, kind="ExternalInput").ap()
    bv = nc.dram_tensor("bv", [GCOLS], F32, kind="ExternalInput").ap()
    out = nc.dram_tensor("out", [S, D], F32, kind="ExternalOutput").ap()

    with tile.TileContext(nc) as tc, ExitStack() as outer:
        const = outer.enter_context(tc.tile_pool(name="const", bufs=1))
        persist = outer.enter_context(tc.tile_pool(name="persist", bufs=1))

        idf = const.tile([128, 128], F32)
        make_identity(nc, idf)
        idr = const.tile([128, 128], F32R)
        nc.vector.tensor_copy(idr, idf)
        ones_f = const.tile([128, 64], F32)
        nc.vector.memset(ones_f, 1.0)
        bq_sb = const.tile([128, NPAIRS], F32)
        bk_sb = const.tile([128, NPAIRS], F32)
        bv_sb = const.tile([128, NPAIRS], F32)
        nc.sync.dma_start(out=bq_sb, in_=bq.rearrange("(p r) -> r p", r=128))
        nc.sync.dma_start(out=bk_sb, in_=bk.rearrange("(p r) -> r p", r=128))
        nc.sync.dma_start(out=bv_sb, in_=bv.rearrange("(p r) -> r p", r=128))

        # x^T resident, rounded to f32r via DVE copy (verifier requirement)
        xT_r = persist.tile([128, DC, S], F32R)
        xT_dram = xT.rearrange("(dc p) n -> p dc n", p=128)

        attnT = [persist.tile([128, S], F32R, name=f"attnT{p}", tag=f"attnT{p}")
                 for p in range(NPAIRS)]

        with ExitStack() as inner:
            xstage = inner.enter_context(tc.tile_pool(name="xstage", bufs=2))
            wstage = inner.enter_context(tc.tile_pool(name="wstage", bufs=1))
            wpool = inner.enter_context(tc.tile_pool(name="wpool", bufs=1))
            qkv = inner.enter_context(tc.tile_pool(name="qkv", bufs=1))
            vpool = inner.enter_context(tc.tile_pool(name="vpool", bufs=1))
            espool = inner.enter_context(tc.tile_pool(name="espool", bufs=5))
            small = inner.enter_context(tc.tile_pool(name="small", bufs=2))
            ps512 = inner.enter_context(
                tc.tile_pool(name="ps512", bufs=2, space="PSUM"))
            pssc = inner.enter_context(
                tc.tile_pool(name="pssc", bufs=2, space="PSUM"))
            psav = inner.enter_context(
                tc.tile_pool(name="psav", bufs=2, space="PSUM"))

            zf = xstage.tile([128, 512], F32, name="zf", tag="zf")
            nc.vector.memset(zf, 0.0)
            zr = xstage.tile([128, 512], F32R, name="zr", tag="zr")
            nc.vector.tensor_copy(zr, zf)
            warm_ps = ps512.tile([128, 512], F32, name="warm_ps",
                                 tag="ps512")
            for _ in range(40):
                nc.tensor.matmul(warm_ps, idr, zr, start=True, stop=True,
                                 skip_group_check=True)

            qeng = [nc.sync, nc.scalar]
            for half in range(4):
                for dc in range(DC):
                    xs = xstage.tile([128, S // 4], F32, name="xs", tag="xs")
                    cols = slice(half * (S // 4), (half + 1) * (S // 4))
                    qeng[(half * DC + dc) % 2].dma_start(
                        out=xs, in_=xT_dram[:, dc, cols])
                    nc.vector.tensor_copy(xT_r[:, dc, cols], xs)

            for p in range(NPAIRS):
                csl = slice(p * 128, (p + 1) * 128)
                # --- projections: Q^T/K^T/V^T pair tiles [128, S]
                pair_t = {}
                for nm, w_ap, b_sb in (("q", wq, bq_sb), ("k", wk, bk_sb),
                                       ("v", wv, bv_sb)):
                    w_r = wpool.tile([128, DC, 128], F32R, name=f"w{nm}_r",
                                     tag=f"w{nm}")
                    wre = w_ap.rearrange("(dc p) m -> p dc m", p=128)
                    for wh in range(2):
                        ws = wstage.tile([128, DC // 2, 128], F32, name="ws",
                                         tag="ws", bufs=2)
                        dsl = slice(wh * (DC // 2), (wh + 1) * (DC // 2))
                        nc.sync.dma_start(out=ws, in_=wre[:, dsl, csl])
                        nc.vector.tensor_copy(w_r[:, dsl, :], ws)
                    t_sb = qkv.tile([128, S], F32R, name=f"{nm}t_sb",
                                    tag=f"{nm}t",
                                    bufs=(1 if nm == "v" else 2))
                    for nt in range(S // 512):
                        mm_ps = ps512.tile([128, 512], F32, name="proj_ps",
                                           tag="ps512")
                        for dc in range(DC):
                            nc.tensor.matmul(
                                mm_ps, w_r[:, dc, :],
                                xT_r[:, dc, nt * 512:(nt + 1) * 512],
                                start=(dc == 0), stop=(dc == DC - 1))
                        nc.scalar.activation(
                            t_sb[:, nt * 512:(nt + 1) * 512], mm_ps,
                            mybir.ActivationFunctionType.Identity,
                            bias=b_sb[:, p:p + 1])
                    pair_t[nm] = t_sb
                qt_sb, kt_sb, vt_sb = pair_t["q"], pair_t["k"], pair_t["v"]

                # --- V^T -> V normal layout [k, 65] per head (ones col last)
                v_sb = vpool.tile([128, NKT, 130], F32R)
                ones3 = ones_f.rearrange("p (a b) -> p a b", b=1)[:, 0:NKT, :]
                nc.vector.tensor_copy(v_sb[:, :, 64:65], ones3)
                nc.vector.tensor_copy(v_sb[:, :, 129:130], ones3)
                for kb in range(NKT):
                    tr_ps = ps512.tile([128, 128], F32R, name="tr_ps",
                                       tag="ps512")
                    nc.tensor.matmul(tr_ps, vt_sb[:, kb * 128:(kb + 1) * 128],
                                     idr, is_transpose=True,
                                     start=True, stop=True)
                    nc.scalar.activation(v_sb[:, kb, 0:64],
                                         tr_ps[:, 0:64],
                                         mybir.ActivationFunctionType.Copy)
                    nc.scalar.activation(v_sb[:, kb, 65:129],
                                         tr_ps[:, 64:128],
                                         mybir.ActivationFunctionType.Copy)

                # --- attention per head
                for hh in range(2):
                    hb = hh * 64
                    vw = slice(hh * 65, hh * 65 + 65)
                    for qt in range(NQT):
                        q0 = qt * 1024
                        av_ps = [psav.tile([65, 512], F32, name=f"av_ps{qh}",
                                           tag="psav") for qh in range(2)]
                        for kt in range(NKT):
                            sc_ps = pssc.tile([128, 1024], F32, name="sc_ps",
                                              tag="pssc")
                            for qh in range(2):
                                nc.tensor.matmul(
                                    sc_ps[:, qh * 512:(qh + 1) * 512],
                                    kt_sb[hb:hb + 64,
                                          kt * 128:(kt + 1) * 128],
                                    qt_sb[hb:hb + 64,
                                          q0 + qh * 512:q0 + (qh + 1) * 512],
                                    start=True, stop=True)
                            es = espool.tile([128, 1024], F32R, name="es",
                                             tag="es")
                            nc.scalar.activation(es, sc_ps, EXP, scale=0.125)
                            for qh in range(2):
                                nc.tensor.matmul(
                                    av_ps[qh], v_sb[:, kt, vw],
                                    es[:, qh * 512:(qh + 1) * 512],
                                    start=(kt == 0), stop=(kt == NKT - 1),
                                    skip_group_check=True)
                        for qh in range(2):
                            col = slice(q0 + qh * 512, q0 + (qh + 1) * 512)
                            av_sb = small.tile([65, 512], F32,
                                               name="av_sb", tag="av_sb")
                            nc.vector.tensor_copy(av_sb, av_ps[qh])
                            bc = small.tile([64, 512], F32, name="bc",
                                            tag="bc", bufs=1)
                            sr = av_sb[64:65, :]
                            rep = bass.AP(tensor=sr.tensor, offset=sr.offset,
                                          ap=[sr.ap[0], [0, 64], [1, 512]])
                            nc.sync.dma_start(out=bc.unsqueeze(1), in_=rep)
                            rec = small.tile([64, 512], F32, name="rec",
                                             tag="rec")
                            nc.vector.reciprocal_approx_fast(out=rec, in_=bc)
                            if hh == 0:
                                nc.vector.tensor_mul(attnT[p][0:64, col],
                                                     av_sb[0:64, :], rec)
                            else:
                                tmp = small.tile([64, 512], F32R, name="tmp",
                                                 tag="tmp", bufs=1)
                                nc.vector.tensor_mul(tmp, av_sb[0:64, :],
                                                     rec)
                                nc.sync.dma_start(out=attnT[p][64:128, col],
                                                  in_=tmp)

        # --- output projection: out[q, :] = sum_p attnT[p]^T @ wo rows
        with ExitStack() as fin:
            wostage = fin.enter_context(tc.tile_pool(name="wostage", bufs=1))
            wopool = fin.enter_context(tc.tile_pool(name="wopool", bufs=1))
            osb = fin.enter_context(tc.tile_pool(name="osb", bufs=4))
            psout = fin.enter_context(
                tc.tile_pool(name="psout", bufs=4, space="PSUM"))
            wo_st = wostage.tile([128, NPAIRS, D], F32)
            nc.sync.dma_start(out=wo_st,
                              in_=wo.rearrange("(p r) n -> r p n", r=128))
            wo_r = wopool.tile([128, NPAIRS, D], F32R)
            nc.vector.tensor_copy(wo_r, wo_st)
            for qc in range(S // 128):
                o_ps = [psout.tile([128, 512], F32, name=f"o_ps{nt}",
                                   tag="psout") for nt in range(2)]
                for p in range(NPAIRS):
                    for nt in range(2):
                        nc.tensor.matmul(
                            o_ps[nt],
                            attnT[p][:, qc * 128:(qc + 1) * 128],
                            wo_r[:, p, nt * 512:(nt + 1) * 512],
                            start=(p == 0), stop=(p == NPAIRS - 1),
                            skip_group_check=True)
                for nt in range(2):
                    o_sb = osb.tile([128, 512], F32, name="o_sb", tag="o_sb")
                    nc.vector.tensor_copy(o_sb, o_ps[nt])
                    nc.sync.dma_start(
                        out=out[qc * 128:(qc + 1) * 128,
                                nt * 512:(nt + 1) * 512],
                        in_=o_sb)

    nc.compile()
    return nc


def _get_compiled():
    global _COMPILED
    if _COMPILED is None:
        _COMPILED = _build()
    return _COMPILED


def make_in_maps(**inputs):
    x = np.asarray(inputs["inputs"], np.float32)
    xTb = [np.ascontiguousarray(x[b].T) for b in range(B)]
    gslice = {}
    for nm in ("Wq", "Wk", "Wv", "Wo", "bq", "bk", "bv"):
        a = np.asarray(inputs[nm], np.float32)
        for g in range(2):
            sl = slice(g * GCOLS, (g + 1) * GCOLS)
            if nm == "Wo":
                gslice[(nm, g)] = np.ascontiguousarray(a[sl, :])
            elif nm.startswith("W"):
                gslice[(nm, g)] = np.ascontiguousarray(a[:, sl])
            else:
                gslice[(nm, g)] = np.ascontiguousarray(a[sl])
    in_maps = []
    for c in range(NCORES):
        g, b = c // B, c % B
        in_maps.append({
            "xT": xTb[b],
            "wq": gslice[("Wq", g)], "wk": gslice[("Wk", g)],
            "wv": gslice[("Wv", g)], "wo": gslice[("Wo", g)],
            "bq": gslice[("bq", g)], "bk": gslice[("bk", g)],
            "bv": gslice[("bv", g)],
        })
    return in_maps


def combine(results, bo):
    out = np.empty((B, S, D), np.float32)
    bo = np.asarray(bo, np.float32)
    for b in range(B):
        out[b] = results[b]["out"] + results[B + b]["out"] + bo
    return out


def kernel(**inputs):
    from concourse import bass_utils
    nc = _get_compiled()
    in_maps = make_in_maps(**inputs)
    res = bass_utils.run_bass_kernel_spmd(
        nc, in_maps, core_ids=list(range(NCORES)))
    return combine(res.results, inputs["bo"])



# revision 3
# speedup vs baseline: 1.0698x; 1.0698x over previous
"""Multi-head attention (B=4, S=2048, D=1024, H=16) on 8 TRN2 NeuronCores.

Sharding: 2D grid (batch x head-group). Core c = g*4 + b handles batch b and
head group g (8 heads = 512 of the 1024 embedding columns).

v2 design (vs baseline): all matmul operands bf16 (rel-err budget allows),
V computed directly in [keys, dh] layout (no PE transposes, no ScalarE
copies), scores for the pair's two heads issued as adjacent row-tiled
matmuls (rows 0-63 / 64-127) so the PE runs them concurrently, ScalarE does
ONLY the exp (one [128,1024] activation per (pair, qt, kt)), every
PSUM eviction is on the DVE, and the V/O biases are folded into the host
combine (softmax rows sum to 1, so  attn(V + bv) = attn(V) + bv  exactly).

Per-core kernel:
  1. x^T [1024, 2048] and all weights DMA'd in bf16, SBUF-resident.
  2. Per pair p (2 heads): Q^T/K^T [128, 2048] = W_p^T @ x^T (PSUM acc over
     8 k-chunks, DVE eviction adds bias, writes bf16).
  3. V for all heads: V[keys, dh] = x @ Wv per 128-key tile (stationary =
     x^T chunk), DVE-evicted into [128, kt, head, 65] with a ones column
     (so P@[V|1] also emits the softmax denominator).
  4. Attention per (pair, 512-query block, key tile): packed scores ->
     one exp -> two attn@V accumulations. Normalization via DMA partition
     broadcast of the denominator row + DVE reciprocal/multiply -> attnT.
  5. Out projection out[q,:] = sum_p attnT[p]^T @ wo rows, interleaved with
     the last pair's attention.
Host sums the two head-group partials per batch and adds bo + bv @ Wo.
"""
import numpy as np

B, S, D, H, DH = 4, 2048, 1024, 16, 64
NCORES = 8
GCOLS = D // 2          # 512 cols per head-group core
NPAIRS = GCOLS // 128   # 4 head-pairs per core
NKT = S // 128          # 16 key tiles
NQT = S // 512          # 4 query blocks of 512
DC = D // 128           # 8 contraction chunks for projections

_COMPILED = None


def _build():
    import concourse.bass as bass
    import concourse.bacc as bacc
    import concourse.tile as tile
    from concourse import mybir
    from contextlib import ExitStack

    F32 = mybir.dt.float32
    BF16 = mybir.dt.bfloat16
    EXP = mybir.ActivationFunctionType.Exp

    nc = bacc.Bacc("TRN2", target_bir_lowering=False, debug=False)
    xT = nc.dram_tensor("xT", [D, S], BF16, kind="ExternalInput").ap()
    wq = nc.dram_tensor("wq", [D, GCOLS], BF16, kind="ExternalInput").ap()
    wk = nc.dram_tensor("wk", [D, GCOLS], BF16, kind="ExternalInput").ap()
    wv = nc.dram_tensor("wv", [D, GCOLS], BF16, kind="ExternalInput").ap()
    wo = nc.dram_tensor("wo", [GCOLS, D], BF16, kind="ExternalInput").ap()
    bq = nc.dram_tensor("bq", [GCOLS], F32, kind="ExternalInput").ap()
    bk = nc.dram_tensor("bk", [GCOLS], F32, kind="ExternalInput").ap()
    out = nc.dram_tensor("out", [S, D], F32, kind="ExternalOutput").ap()

    with tile.TileContext(nc) as tc, ExitStack() as st:
        const = st.enter_context(tc.tile_pool(name="const", bufs=1))
        persist = st.enter_context(tc.tile_pool(name="persist", bufs=1))
        qkpool = st.enter_context(tc.tile_pool(name="qkpool", bufs=2))
        espool = st.enter_context(tc.tile_pool(name="espool", bufs=4))
        small = st.enter_context(tc.tile_pool(name="small", bufs=3))
        osb = st.enter_context(tc.tile_pool(name="osb", bufs=3))
        pssc = st.enter_context(
            tc.tile_pool(name="pssc", bufs=2, space="PSUM"))
        psav = st.enter_context(
            tc.tile_pool(name="psav", bufs=2, space="PSUM"))
        pspj = st.enter_context(
            tc.tile_pool(name="pspj", bufs=2, space="PSUM"))

        # --- biases (per-partition scalars: partition r = within-pair dim,
        # col p = pair index)
        bq_sb = const.tile([128, NPAIRS], F32)
        bk_sb = const.tile([128, NPAIRS], F32)
        nc.sync.dma_start(out=bq_sb, in_=bq.rearrange("(p r) -> r p", r=128))
        nc.sync.dma_start(out=bk_sb, in_=bk.rearrange("(p r) -> r p", r=128))

        # --- resident inputs (bf16, DMA'd directly, no conversion pass)
        xT_sb = persist.tile([128, DC, S], BF16, name="xT_sb")
        xT_dram = xT.rearrange("(dc p) n -> p dc n", p=128)
        wq_sb = persist.tile([128, DC, GCOLS], BF16, name="wq_sb")
        wk_sb = persist.tile([128, DC, GCOLS], BF16, name="wk_sb")
        wv_sb = persist.tile([128, DC, GCOLS], BF16, name="wv_sb")
        wo_sb = persist.tile([128, NPAIRS, D], BF16, name="wo_sb")
        nc.scalar.dma_start(out=wq_sb,
                            in_=wq.rearrange("(dc p) m -> p dc m", p=128))
        nc.scalar.dma_start(out=wk_sb,
                            in_=wk.rearrange("(dc p) m -> p dc m", p=128))
        for dc in range(DC):
            nc.sync.dma_start(out=xT_sb[:, dc, :], in_=xT_dram[:, dc, :])
        nc.scalar.dma_start(out=wv_sb,
                            in_=wv.rearrange("(dc p) m -> p dc m", p=128))
        nc.scalar.dma_start(out=wo_sb,
                            in_=wo.rearrange("(p r) n -> r p n", r=128))

        # --- V in [key, dh] layout, ones column at dh=64 per head
        v_sb = persist.tile([128, NKT, 8, 65], BF16, name="v_sb")
        nc.vector.memset(v_sb[:, :, :, 64:65], 1.0)

        # --- attention outputs (transposed), bf16 for the out-projection
        attnT = [persist.tile([128, S], BF16, name=f"attnT{p}",
                              tag=f"attnT{p}") for p in range(NPAIRS)]

        # --- HAM warmup: dummy matmuls so the PE reaches K=8/8 while the
        # input DMAs run
        zw = const.tile([128, 128], BF16)
        zf = const.tile([128, 512], BF16)
        nc.vector.memset(zw, 0.0)
        nc.vector.memset(zf, 0.0)
        warm_ps = pspj.tile([128, 512], F32, name="warm_ps", tag="pspj")
        for _ in range(56):
            nc.tensor.matmul(warm_ps, zw, zf, start=True, stop=True,
                             skip_group_check=True)

        qk_tiles = {}

        def proj_group(p, mat, nt):
            """One 512-col block of the Q^T/K^T projection for pair p."""
            if (p, mat) not in qk_tiles:
                qk_tiles[(p, mat)] = qkpool.tile(
                    [128, S], BF16, name=f"{mat}t{p}", tag=f"{mat}t")
            t_sb = qk_tiles[(p, mat)]
            w_sb, b_sb = (wq_sb, bq_sb) if mat == "q" else (wk_sb, bk_sb)
            csl = slice(p * 128, (p + 1) * 128)
            ps = pspj.tile([128, 512], F32, name="proj_ps", tag="pspj")
            for dc in range(DC):
                nc.tensor.matmul(ps, w_sb[:, dc, csl],
                                 xT_sb[:, dc, nt * 512:(nt + 1) * 512],
                                 start=(dc == 0), stop=(dc == DC - 1))
            nc.vector.tensor_scalar_add(t_sb[:, nt * 512:(nt + 1) * 512],
                                        ps, b_sb[:, p:p + 1])

        def v_group(p, ktile):
            """One 128-key tile of V for pair p (cols 2p*64 .. 2p*64+127)."""
            csl = slice(p * 128, (p + 1) * 128)
            ps = pspj.tile([128, 128], F32, name="v_ps", tag="pspj")
            for dc in range(DC):
                nc.tensor.matmul(ps, xT_sb[:, dc, ktile * 128:(ktile + 1) * 128],
                                 wv_sb[:, dc, csl],
                                 start=(dc == 0), stop=(dc == DC - 1))
            nc.vector.tensor_copy(v_sb[:, ktile, 2 * p:2 * p + 2, 0:64], ps)

        def pair_work(p):
            """All prep work for pair p as a list of thunks."""
            work = []
            for nt in range(4):
                work.append(lambda nt=nt: proj_group(p, "k", nt))
            for nt in range(4):
                work.append(lambda nt=nt: proj_group(p, "q", nt))
            for ktile in range(NKT):
                work.append(lambda kt=ktile: v_group(p, kt))
            return work

        def attention(p, qt, hook):
            q0 = qt * 512
            qt_sb = qk_tiles[(p, "q")]
            kt_sb = qk_tiles[(p, "k")]
            av = [psav.tile([65, 512], F32, name=f"av{hh}", tag="psav")
                  for hh in range(2)]
            for kt in range(NKT):
                ps = pssc.tile([128, 1024], F32, name="sc_ps", tag="pssc")
                for hh in range(2):
                    hb = hh * 64
                    nc.tensor.matmul(
                        ps[:, hh * 512:(hh + 1) * 512],
                        kt_sb[hb:hb + 64, kt * 128:(kt + 1) * 128],
                        qt_sb[hb:hb + 64, q0:q0 + 512],
                        start=True, stop=True)
                es = espool.tile([128, 1024], BF16, name="es", tag="es")
                nc.scalar.activation(es, ps, EXP, scale=0.125)
                for hh in range(2):
                    nc.tensor.matmul(
                        av[hh], v_sb[:, kt, 2 * p + hh, :],
                        es[:, hh * 512:(hh + 1) * 512],
                        start=(kt == 0), stop=(kt == NKT - 1),
                        skip_group_check=True)
                hook(kt)
            for hh in range(2):
                av_sb = small.tile([65, 512], F32, name="av_sb", tag="av_sb")
                nc.vector.tensor_copy(av_sb, av[hh])
                bc = small.tile([64, 512], F32, name="bc", tag="bc", bufs=2)
                sr = av_sb[64:65, :]
                rep = bass.AP(tensor=sr.tensor, offset=sr.offset,
                              ap=[sr.ap[0], [0, 64], [1, 512]])
                nc.gpsimd.dma_start(out=bc.unsqueeze(1), in_=rep)
                rec = small.tile([64, 512], F32, name="rec", tag="rec",
                                 bufs=2)
                nc.vector.reciprocal_approx_fast(out=rec, in_=bc)
                if hh == 0:
                    nc.vector.tensor_mul(attnT[p][0:64, q0:q0 + 512],
                                         av_sb[0:64, :], rec)
                else:
                    # engines can't shift partitions; route via DMA
                    tmp = small.tile([64, 512], BF16, name="tmp", tag="tmp",
                                     bufs=2)
                    nc.vector.tensor_mul(tmp, av_sb[0:64, :], rec)
                    nc.gpsimd.dma_start(out=attnT[p][64:128, q0:q0 + 512],
                                        in_=tmp)

        def outproj(qt):
            for qc in range(qt * 4, (qt + 1) * 4):
                o_ps = [pspj.tile([128, 512], F32, name=f"o_ps{nt}",
                                  tag="pspj") for nt in range(2)]
                for pp in range(NPAIRS):
                    for nt in range(2):
                        nc.tensor.matmul(
                            o_ps[nt], attnT[pp][:, qc * 128:(qc + 1) * 128],
                            wo_sb[:, pp, nt * 512:(nt + 1) * 512],
                            start=(pp == 0), stop=(pp == NPAIRS - 1),
                            skip_group_check=True)
                for nt in range(2):
                    o_sb = osb.tile([128, 512], F32, name="o_sb", tag="o_sb")
                    nc.vector.tensor_copy(o_sb, o_ps[nt])
                    nc.gpsimd.dma_start(
                        out=out[qc * 128:(qc + 1) * 128,
                                nt * 512:(nt + 1) * 512],
                        in_=o_sb)

        # --- main flow: pair-major attention, next pair's projections and V
        # interleaved into the (ScalarE-bound) attention key-tile loop
        for w in pair_work(0):
            w()
        for p in range(NPAIRS):
            pending = pair_work(p + 1) if p + 1 < NPAIRS else []
            # spread the 24 prep groups over the 64 kt slots of 4 qt blocks
            for qt in range(NQT):
                chunk = pending[qt * 6:(qt + 1) * 6]

                def hook(kt, chunk=chunk):
                    if kt % 2 == 1 and kt // 2 < len(chunk):
                        chunk[kt // 2]()
                attention(p, qt, hook)
                if p == NPAIRS - 1:
                    outproj(qt)

    nc.compile()
    return nc


def _get_compiled():
    global _COMPILED
    if _COMPILED is None:
        _COMPILED = _build()
    return _COMPILED


def make_in_maps(**inputs):
    import ml_dtypes
    bf16 = ml_dtypes.bfloat16
    x = np.asarray(inputs["inputs"], np.float32)
    xTb = [np.ascontiguousarray(x[b].T).astype(bf16) for b in range(B)]
    gslice = {}
    for nm in ("Wq", "Wk", "Wv", "Wo", "bq", "bk"):
        a = np.asarray(inputs[nm], np.float32)
        for g in range(2):
            sl = slice(g * GCOLS, (g + 1) * GCOLS)
            if nm == "Wo":
                gslice[(nm, g)] = np.ascontiguousarray(a[sl, :]).astype(bf16)
            elif nm.startswith("W"):
                gslice[(nm, g)] = np.ascontiguousarray(a[:, sl]).astype(bf16)
            else:
                gslice[(nm, g)] = np.ascontiguousarray(a[sl])
    in_maps = []
    for c in range(NCORES):
        g, b = c // B, c % B
        in_maps.append({
            "xT": xTb[b],
            "wq": gslice[("Wq", g)], "wk": gslice[("Wk", g)],
            "wv": gslice[("Wv", g)], "wo": gslice[("Wo", g)],
            "bq": gslice[("bq", g)], "bk": gslice[("bk", g)],
        })
    return in_maps


def combine(results, bo, bv, Wo):
    out = np.empty((B, S, D), np.float32)
    bo = np.asarray(bo, np.float32)
    bv = np.asarray(bv, np.float32)
    Wo = np.asarray(Wo, np.float32)
    const_row = bo + bv @ Wo
    for b in range(B):
        out[b] = results[b]["out"] + results[B + b]["out"] + const_row
    return out


def kernel(**inputs):
    from concourse import bass_utils
    nc = _get_compiled()
    in_maps = make_in_maps(**inputs)
    res = bass_utils.run_bass_kernel_spmd(
        nc, in_maps, core_ids=list(range(NCORES)))
    return combine(res.results, inputs["bo"], inputs["bv"], inputs["Wo"])


# revision 9
# speedup vs baseline: 1.1447x; 1.0701x over previous
"""Multi-head attention (B=4, S=2048, D=1024, H=16) on 8 TRN2 NeuronCores.

Sharding: 2D grid (batch x head-group). Core c = g*4 + b handles batch b and
head group g (8 heads = 512 of the 1024 embedding columns).

v3 design: all matmul operands bf16, V computed directly in [keys, dh]
layout with one N=512 matmul group per 128-key tile covering all 4 head
pairs, scores for a pair's two heads issued as adjacent row-tiled matmuls
(rows 0-63 / 64-127, concurrent on the PE), ScalarE does ONLY the exp
(one [128,1024] activation per (pair, qt, kt)), every PSUM eviction on the
DVE, softmax-denominator broadcast on the Pool engine (partition_broadcast)
instead of a DMA round trip, and V/O biases folded into the host combine
(softmax rows sum to 1, so attn(V + bv) = attn(V) + bv exactly).

Loop structure: pair-major attention over 4 query blocks of 512; the
key-tile loop is ScalarE(exp)-bound, so the next pair's Q/K projections
(and, during the first block, the V tiles) are interleaved into its spare
PE slots. The out-projection runs per query block during the last pair's
attention. Host sums the two head-group partials and adds bo + bv @ Wo.
"""
import numpy as np

B, S, D, H, DH = 4, 2048, 1024, 16, 64
NCORES = 8
GCOLS = D // 2          # 512 cols per head-group core
NPAIRS = GCOLS // 128   # 4 head-pairs per core
NKT = S // 128          # 16 key tiles
NQT = S // 512          # 4 query blocks of 512
DC = D // 128           # 8 contraction chunks for projections

_COMPILED = None


def _build():
    import concourse.bass as bass
    import concourse.bacc as bacc
    import concourse.tile as tile
    from concourse import mybir
    from contextlib import ExitStack

    F32 = mybir.dt.float32
    BF16 = mybir.dt.bfloat16
    EXP = mybir.ActivationFunctionType.Exp

    nc = bacc.Bacc("TRN2", target_bir_lowering=False, debug=False)
    xT = nc.dram_tensor("xT", [D, S], BF16, kind="ExternalInput").ap()
    wq = nc.dram_tensor("wq", [D, GCOLS], BF16, kind="ExternalInput").ap()
    wk = nc.dram_tensor("wk", [D, GCOLS], BF16, kind="ExternalInput").ap()
    wv = nc.dram_tensor("wv", [D, GCOLS], BF16, kind="ExternalInput").ap()
    wo = nc.dram_tensor("wo", [GCOLS, D], BF16, kind="ExternalInput").ap()
    bq = nc.dram_tensor("bq", [GCOLS], F32, kind="ExternalInput").ap()
    bk = nc.dram_tensor("bk", [GCOLS], F32, kind="ExternalInput").ap()
    out = nc.dram_tensor("out", [S, D], F32, kind="ExternalOutput").ap()

    with tile.TileContext(nc) as tc, ExitStack() as st:
        const = st.enter_context(tc.tile_pool(name="const", bufs=1))
        persist = st.enter_context(tc.tile_pool(name="persist", bufs=1))
        qkpool = st.enter_context(tc.tile_pool(name="qkpool", bufs=2))
        espool = st.enter_context(tc.tile_pool(name="espool", bufs=6))
        small = st.enter_context(tc.tile_pool(name="small", bufs=3))
        osb = st.enter_context(tc.tile_pool(name="osb", bufs=4))
        pssc = st.enter_context(
            tc.tile_pool(name="pssc", bufs=2, space="PSUM"))
        psav = st.enter_context(
            tc.tile_pool(name="psav", bufs=2, space="PSUM"))
        pspj = st.enter_context(
            tc.tile_pool(name="pspj", bufs=2, space="PSUM"))

        # --- biases (per-partition scalars: partition r = within-pair dim,
        # col p = pair index)
        bq_sb = const.tile([128, NPAIRS], F32)
        bk_sb = const.tile([128, NPAIRS], F32)
        nc.gpsimd.dma_start(out=bq_sb, in_=bq.rearrange("(p r) -> r p", r=128))
        nc.gpsimd.dma_start(out=bk_sb, in_=bk.rearrange("(p r) -> r p", r=128))

        # --- resident inputs (bf16, DMA'd directly on 4 queues)
        xT_sb = persist.tile([128, DC, S], BF16, name="xT_sb")
        xT_dram = xT.rearrange("(dc p) n -> p dc n", p=128)
        wq_sb = persist.tile([128, DC, GCOLS], BF16, name="wq_sb")
        wk_sb = persist.tile([128, DC, GCOLS], BF16, name="wk_sb")
        wv_sb = persist.tile([128, DC, GCOLS], BF16, name="wv_sb")
        wo_sb = persist.tile([128, NPAIRS, D], BF16, name="wo_sb")
        nc.scalar.dma_start(out=wq_sb,
                            in_=wq.rearrange("(dc p) m -> p dc m", p=128))
        nc.scalar.dma_start(out=wk_sb,
                            in_=wk.rearrange("(dc p) m -> p dc m", p=128))
        for dc in range(DC):
            eng = nc.sync if dc % 2 == 0 else nc.gpsimd
            eng.dma_start(out=xT_sb[:, dc, :], in_=xT_dram[:, dc, :])
        nc.scalar.dma_start(out=wv_sb,
                            in_=wv.rearrange("(dc p) m -> p dc m", p=128))
        nc.scalar.dma_start(out=wo_sb,
                            in_=wo.rearrange("(p r) n -> r p n", r=128))

        # --- V in [key, dh] layout, ones column at dh=64 per head
        v_sb = persist.tile([128, NKT, 8, 65], BF16, name="v_sb")
        nc.vector.memset(v_sb[:, :, :, 64:65], 1.0)

        # --- attention outputs (transposed), bf16 for the out-projection
        attnT = [persist.tile([128, S], BF16, name=f"attnT{p}",
                              tag=f"attnT{p}") for p in range(NPAIRS)]

        # --- HAM warmup: dummy matmuls so the PE reaches K=8/8 while the
        # input DMAs run
        zw = const.tile([128, 128], BF16)
        zf = const.tile([128, 512], BF16)
        nc.vector.memset(zw, 0.0)
        nc.vector.memset(zf, 0.0)
        warm_ps = pspj.tile([128, 512], F32, name="warm_ps", tag="pspj")
        for _ in range(32):
            nc.tensor.matmul(warm_ps, zw, zf, start=True, stop=True,
                             skip_group_check=True)

        qk_tiles = {}

        def proj_group(p, mat, nt):
            """One 512-col block of the Q^T/K^T projection for pair p."""
            if (p, mat) not in qk_tiles:
                qk_tiles[(p, mat)] = qkpool.tile(
                    [128, S], BF16, name=f"{mat}t{p}", tag=f"{mat}t")
            t_sb = qk_tiles[(p, mat)]
            w_sb, b_sb = (wq_sb, bq_sb) if mat == "q" else (wk_sb, bk_sb)
            csl = slice(p * 128, (p + 1) * 128)
            ps = pspj.tile([128, 512], F32, name="proj_ps", tag="pspj")
            for dc in range(DC):
                nc.tensor.matmul(ps, w_sb[:, dc, csl],
                                 xT_sb[:, dc, nt * 512:(nt + 1) * 512],
                                 start=(dc == 0), stop=(dc == DC - 1))
            nc.vector.tensor_scalar_add(t_sb[:, nt * 512:(nt + 1) * 512],
                                        ps, b_sb[:, p:p + 1])

        def v_group(ktile):
            """One 128-key tile of V for ALL 4 pairs (N=512)."""
            ps = pspj.tile([128, 512], F32, name="v_ps", tag="pspj")
            for dc in range(DC):
                nc.tensor.matmul(ps, xT_sb[:, dc, ktile * 128:(ktile + 1) * 128],
                                 wv_sb[:, dc, :],
                                 start=(dc == 0), stop=(dc == DC - 1))
            nc.vector.tensor_copy(v_sb[:, ktile, :, 0:64], ps)

        def attention(p, qt, hook):
            q0 = qt * 512
            qt_sb = qk_tiles[(p, "q")]
            kt_sb = qk_tiles[(p, "k")]
            av = [psav.tile([65, 512], F32, name=f"av{hh}", tag="psav")
                  for hh in range(2)]
            for kt in range(NKT):
                ps = pssc.tile([128, 1024], F32, name="sc_ps", tag="pssc")
                for hh in range(2):
                    hb = hh * 64
                    nc.tensor.matmul(
                        ps[:, hh * 512:(hh + 1) * 512],
                        kt_sb[hb:hb + 64, kt * 128:(kt + 1) * 128],
                        qt_sb[hb:hb + 64, q0:q0 + 512],
                        start=True, stop=True)
                es = espool.tile([128, 1024], BF16, name="es", tag="es")
                nc.scalar.activation(es, ps, EXP, scale=0.125)
                # prep work for the next pair / V tiles is issued BEFORE the
                # attn@V matmuls so anything they depend on is ahead of them
                # in the PE queue
                hook(kt)
                for hh in range(2):
                    nc.tensor.matmul(
                        av[hh], v_sb[:, kt, 2 * p + hh, :],
                        es[:, hh * 512:(hh + 1) * 512],
                        start=(kt == 0), stop=(kt == NKT - 1),
                        skip_group_check=True)
            for hh in range(2):
                av_sb = small.tile([65, 512], F32, name="av_sb", tag="av_sb")
                nc.vector.tensor_copy(av_sb, av[hh])
                bc = small.tile([64, 512], F32, name="bc", tag="bc", bufs=2)
                sr = av_sb[64:65, :]
                rep = bass.AP(tensor=sr.tensor, offset=sr.offset,
                              ap=[sr.ap[0], [0, 64], [1, 512]])
                nc.sync.dma_start(out=bc.unsqueeze(1), in_=rep)
                rec = small.tile([64, 512], F32, name="rec", tag="rec",
                                 bufs=2)
                nc.vector.reciprocal_approx_fast(out=rec, in_=bc)
                if hh == 0:
                    nc.vector.tensor_mul(attnT[p][0:64, q0:q0 + 512],
                                         av_sb[0:64, :], rec)
                else:
                    # engines can't shift partitions; route via DMA
                    tmp = small.tile([64, 512], BF16, name="tmp", tag="tmp",
                                     bufs=2)
                    nc.vector.tensor_mul(tmp, av_sb[0:64, :], rec)
                    nc.sync.dma_start(out=attnT[p][64:128, q0:q0 + 512],
                                      in_=tmp)

        def outproj(qt):
            for qc in range(qt * 4, (qt + 1) * 4):
                o_ps = [pspj.tile([128, 512], F32, name=f"o_ps{nt}",
                                  tag="pspj") for nt in range(2)]
                for pp in range(NPAIRS):
                    for nt in range(2):
                        nc.tensor.matmul(
                            o_ps[nt], attnT[pp][:, qc * 128:(qc + 1) * 128],
                            wo_sb[:, pp, nt * 512:(nt + 1) * 512],
                            start=(pp == 0), stop=(pp == NPAIRS - 1),
                            skip_group_check=True)
                for nt in range(2):
                    o_sb = osb.tile([128, 512], F32, name="o_sb", tag="o_sb")
                    nc.vector.tensor_copy(o_sb, o_ps[nt])
                    nc.sync.dma_start(
                        out=out[qc * 128:(qc + 1) * 128,
                                nt * 512:(nt + 1) * 512],
                        in_=o_sb)

        # --- main flow ----------------------------------------------------
        # pair 0 prep: K fully, then Q (qt0 block first); V tiles 0-2 ahead
        # of the attention loop, the rest interleaved per key tile
        for nt in range(4):
            proj_group(0, "k", nt)
        for nt in range(4):
            proj_group(0, "q", nt)
        for ktile in range(3):
            v_group(ktile)

        for p in range(NPAIRS):
            pending = []
            if p == 0:
                # remaining V tiles during qt0, ahead of their consumers
                pending.append([lambda kt=k: v_group(kt + 3)
                                for k in range(NKT - 3)] + [None] * 3)
            else:
                pending.append([None] * NKT)
            # next pair's projections spread over qt1..qt3
            if p + 1 < NPAIRS:
                nxt = ([lambda nt=nt: proj_group(p + 1, "k", nt)
                        for nt in range(4)] +
                       [lambda: proj_group(p + 1, "q", 0)])
                nxt2 = [lambda nt=nt: proj_group(p + 1, "q", nt)
                        for nt in range(1, 4)]
                pending += [nxt + [None] * (NKT - len(nxt)),
                            nxt2 + [None] * (NKT - len(nxt2)),
                            [None] * NKT]
            else:
                pending += [[None] * NKT] * 3

            for qt in range(NQT):
                sched = pending[qt]

                def hook(kt, sched=sched):
                    w = sched[kt] if kt < len(sched) else None
                    if w is not None:
                        w()
                attention(p, qt, hook)
                if p == NPAIRS - 1:
                    outproj(qt)

    nc.compile()
    return nc


def _get_compiled():
    global _COMPILED
    if _COMPILED is None:
        _COMPILED = _build()
    return _COMPILED


def make_in_maps(**inputs):
    import ml_dtypes
    bf16 = ml_dtypes.bfloat16
    x = np.asarray(inputs["inputs"], np.float32)
    xTb = [np.ascontiguousarray(x[b].T).astype(bf16) for b in range(B)]
    gslice = {}
    for nm in ("Wq", "Wk", "Wv", "Wo", "bq", "bk"):
        a = np.asarray(inputs[nm], np.float32)
        for g in range(2):
            sl = slice(g * GCOLS, (g + 1) * GCOLS)
            if nm == "Wo":
                gslice[(nm, g)] = np.ascontiguousarray(a[sl, :]).astype(bf16)
            elif nm.startswith("W"):
                gslice[(nm, g)] = np.ascontiguousarray(a[:, sl]).astype(bf16)
            else:
                gslice[(nm, g)] = np.ascontiguousarray(a[sl])
    in_maps = []
    for c in range(NCORES):
        g, b = c // B, c % B
        in_maps.append({
            "xT": xTb[b],
            "wq": gslice[("Wq", g)], "wk": gslice[("Wk", g)],
            "wv": gslice[("Wv", g)], "wo": gslice[("Wo", g)],
            "bq": gslice[("bq", g)], "bk": gslice[("bk", g)],
        })
    return in_maps


def combine(results, bo, bv, Wo):
    out = np.empty((B, S, D), np.float32)
    bo = np.asarray(bo, np.float32)
    bv = np.asarray(bv, np.float32)
    Wo = np.asarray(Wo, np.float32)
    const_row = bo + bv @ Wo
    for b in range(B):
        out[b] = results[b]["out"] + results[B + b]["out"] + const_row
    return out


def kernel(**inputs):
    from concourse import bass_utils
    nc = _get_compiled()
    in_maps = make_in_maps(**inputs)
    res = bass_utils.run_bass_kernel_spmd(
        nc, in_maps, core_ids=list(range(NCORES)))
    return combine(res.results, inputs["bo"], inputs["bv"], inputs["Wo"])


# revision 12
# speedup vs baseline: 1.1455x; 1.0007x over previous
"""Multi-head attention (B=4, S=2048, D=1024, H=16) on 8 TRN2 NeuronCores.

Sharding: 2D grid (batch x head-group). Core c = g*4 + b handles batch b and
head group g (8 heads = 512 of the 1024 embedding columns).

v4 design: all matmul operands bf16. V computed directly in [keys, dh]
layout (one N=512 matmul group per 128-key tile covering all 4 head pairs).
Scores for a pair's two heads issue as adjacent row-tiled matmuls
(rows 0-63 / 64-127, concurrent on the PE). ScalarE does ONLY the exp (one
[128,1024] activation per (pair, qt, kt)) — it is the bottleneck engine, so
everything else is kept off it. All PSUM evictions on the DVE. V/O biases
are folded into the host combine (softmax rows sum to 1, so
attn(V + bv) = attn(V) + bv exactly).

Scheduling: the key-tile loop of each (pair, 512-query block) is
exp-bound; its spare PE/DVE slots absorb, via a work queue consumed one
item per key tile: the previous block's softmax normalization (whose
denominator partition-broadcast rides a DMA round trip — deferring it
keeps the DVE's in-order queue from blocking PSUM evictions), the next
pair's Q/K projection groups, the V tiles (first block only), and the
out-projection of the previous query block (last pair only). x^T is DMA'd
in 512-column slices so the first projections start as soon as ~1MB has
landed. Host sums the two head-group partials and adds bo + bv @ Wo.
"""
import numpy as np

B, S, D, H, DH = 4, 2048, 1024, 16, 64
NCORES = 8
GCOLS = D // 2          # 512 cols per head-group core
NPAIRS = GCOLS // 128   # 4 head-pairs per core
NKT = S // 128          # 16 key tiles
NQT = S // 512          # 4 query blocks of 512
DC = D // 128           # 8 contraction chunks for projections

_COMPILED = None


def _build():
    import concourse.bass as bass
    import concourse.bacc as bacc
    import concourse.tile as tile
    from concourse import mybir
    from contextlib import ExitStack

    F32 = mybir.dt.float32
    BF16 = mybir.dt.bfloat16
    EXP = mybir.ActivationFunctionType.Exp

    nc = bacc.Bacc("TRN2", target_bir_lowering=False, debug=False)
    xT = nc.dram_tensor("xT", [D, S], BF16, kind="ExternalInput").ap()
    wq = nc.dram_tensor("wq", [D, GCOLS], BF16, kind="ExternalInput").ap()
    wk = nc.dram_tensor("wk", [D, GCOLS], BF16, kind="ExternalInput").ap()
    wv = nc.dram_tensor("wv", [D, GCOLS], BF16, kind="ExternalInput").ap()
    wo = nc.dram_tensor("wo", [GCOLS, D], BF16, kind="ExternalInput").ap()
    bq = nc.dram_tensor("bq", [GCOLS], F32, kind="ExternalInput").ap()
    bk = nc.dram_tensor("bk", [GCOLS], F32, kind="ExternalInput").ap()
    out = nc.dram_tensor("out", [S, D], F32, kind="ExternalOutput").ap()

    with tile.TileContext(nc) as tc, ExitStack() as st:
        const = st.enter_context(tc.tile_pool(name="const", bufs=1))
        persist = st.enter_context(tc.tile_pool(name="persist", bufs=1))
        qkpool = st.enter_context(tc.tile_pool(name="qkpool", bufs=2))
        espool = st.enter_context(tc.tile_pool(name="espool", bufs=6))
        small = st.enter_context(tc.tile_pool(name="small", bufs=4))
        osb = st.enter_context(tc.tile_pool(name="osb", bufs=4))
        pssc = st.enter_context(
            tc.tile_pool(name="pssc", bufs=2, space="PSUM"))
        psav = st.enter_context(
            tc.tile_pool(name="psav", bufs=2, space="PSUM"))
        pspj = st.enter_context(
            tc.tile_pool(name="pspj", bufs=2, space="PSUM"))

        # --- warmup constants first so the DVE memsets clear quickly
        zw = const.tile([128, 128], BF16)
        zf = const.tile([128, 512], BF16)
        nc.vector.memset(zw, 0.0)
        nc.vector.memset(zf, 0.0)

        # --- biases (per-partition scalars: partition r = within-pair dim,
        # col p = pair index)
        bq_sb = const.tile([128, NPAIRS], F32)
        bk_sb = const.tile([128, NPAIRS], F32)
        nc.sync.dma_start(out=bq_sb, in_=bq.rearrange("(p r) -> r p", r=128))
        nc.sync.dma_start(out=bk_sb, in_=bk.rearrange("(p r) -> r p", r=128))

        # --- resident inputs (bf16). x^T lands in 512-column slices,
        # nt-major, so the first projection groups can start early.
        xT_sb = persist.tile([128, DC, S], BF16, name="xT_sb")
        xT_dram = xT.rearrange("(dc p) n -> p dc n", p=128)
        wq_sb = persist.tile([128, DC, GCOLS], BF16, name="wq_sb")
        wk_sb = persist.tile([128, DC, GCOLS], BF16, name="wk_sb")
        wv_sb = persist.tile([128, DC, GCOLS], BF16, name="wv_sb")
        wo_sb = persist.tile([128, NPAIRS, D], BF16, name="wo_sb")
        nc.scalar.dma_start(out=wk_sb,
                            in_=wk.rearrange("(dc p) m -> p dc m", p=128))
        for nt in range(4):
            cols = slice(nt * 512, (nt + 1) * 512)
            for dch in range(2):
                dsl = slice(dch * 4, (dch + 1) * 4)
                eng = nc.sync if dch == 0 else nc.gpsimd
                eng.dma_start(out=xT_sb[:, dsl, cols],
                              in_=xT_dram[:, dsl, cols])
        nc.scalar.dma_start(out=wq_sb,
                            in_=wq.rearrange("(dc p) m -> p dc m", p=128))
        nc.gpsimd.dma_start(out=wv_sb,
                            in_=wv.rearrange("(dc p) m -> p dc m", p=128))
        nc.scalar.dma_start(out=wo_sb,
                            in_=wo.rearrange("(p r) n -> r p n", r=128))

        # --- V in [key, dh] layout, ones column at dh=64 per head
        v_sb = persist.tile([128, NKT, 8, 65], BF16, name="v_sb")
        nc.vector.memset(v_sb[:, :, :, 64:65], 1.0)

        # --- attention outputs (transposed), bf16 for the out-projection
        attnT = [persist.tile([128, S], BF16, name=f"attnT{p}",
                              tag=f"attnT{p}") for p in range(NPAIRS)]

        # --- HAM warmup while the input DMAs land
        warm_ps = pspj.tile([128, 512], F32, name="warm_ps", tag="pspj")
        for _ in range(24):
            nc.tensor.matmul(warm_ps, zw, zf, start=True, stop=True,
                             skip_group_check=True)

        qk_tiles = {}

        def proj_group(p, mat, nt):
            """One 512-col block of the Q^T/K^T projection for pair p."""
            if (p, mat) not in qk_tiles:
                qk_tiles[(p, mat)] = qkpool.tile(
                    [128, S], BF16, name=f"{mat}t{p}", tag=f"{mat}t")
            t_sb = qk_tiles[(p, mat)]
            w_sb, b_sb = (wq_sb, bq_sb) if mat == "q" else (wk_sb, bk_sb)
            csl = slice(p * 128, (p + 1) * 128)
            ps = pspj.tile([128, 512], F32, name="proj_ps", tag="pspj")
            for dc in range(DC):
                nc.tensor.matmul(ps, w_sb[:, dc, csl],
                                 xT_sb[:, dc, nt * 512:(nt + 1) * 512],
                                 start=(dc == 0), stop=(dc == DC - 1))
            nc.vector.tensor_scalar_add(t_sb[:, nt * 512:(nt + 1) * 512],
                                        ps, b_sb[:, p:p + 1])

        def v_group(ktile):
            """One 128-key tile of V for ALL 4 pairs (N=512)."""
            ps = pspj.tile([128, 512], F32, name="v_ps", tag="pspj")
            for dc in range(DC):
                nc.tensor.matmul(ps, xT_sb[:, dc, ktile * 128:(ktile + 1) * 128],
                                 wv_sb[:, dc, :],
                                 start=(dc == 0), stop=(dc == DC - 1))
            nc.vector.tensor_copy(v_sb[:, ktile, :, 0:64], ps)

        def attention(p, qt, work):
            """kt loop; pops one thunk from `work` per key tile. Returns
            deferred normalization closures."""
            q0 = qt * 512
            qt_sb = qk_tiles[(p, "q")]
            kt_sb = qk_tiles[(p, "k")]
            av = [psav.tile([65, 512], F32, name=f"av{hh}", tag="psav")
                  for hh in range(2)]
            for kt in range(NKT):
                ps = pssc.tile([128, 1024], F32, name="sc_ps", tag="pssc")
                for hh in range(2):
                    hb = hh * 64
                    nc.tensor.matmul(
                        ps[:, hh * 512:(hh + 1) * 512],
                        kt_sb[hb:hb + 64, kt * 128:(kt + 1) * 128],
                        qt_sb[hb:hb + 64, q0:q0 + 512],
                        start=True, stop=True)
                es = espool.tile([128, 1024], BF16, name="es", tag="es")
                nc.scalar.activation(es, ps, EXP, scale=0.125)
                # deferred/prep work BEFORE the attn@V matmuls so anything
                # they depend on is ahead of them in the engine queues
                if work:
                    work.pop(0)()
                for hh in range(2):
                    nc.tensor.matmul(
                        av[hh], v_sb[:, kt, 2 * p + hh, :],
                        es[:, hh * 512:(hh + 1) * 512],
                        start=(kt == 0), stop=(kt == NKT - 1),
                        skip_group_check=True)
            # prompt part: free the PSUM accumulators, launch the
            # denominator broadcast DMAs
            fin = []
            for hh in range(2):
                av_sb = small.tile([65, 512], F32, name="av_sb", tag="av_sb")
                nc.vector.tensor_copy(av_sb, av[hh])
                bc = small.tile([64, 512], F32, name="bc", tag="bc")
                sr = av_sb[64:65, :]
                rep = bass.AP(tensor=sr.tensor, offset=sr.offset,
                              ap=[sr.ap[0], [0, 64], [1, 512]])
                nc.sync.dma_start(out=bc.unsqueeze(1), in_=rep)

                def finish(hh=hh, av_sb=av_sb, bc=bc):
                    rec = small.tile([64, 512], F32, name="rec", tag="rec")
                    nc.vector.reciprocal_approx_fast(out=rec, in_=bc)
                    if hh == 0:
                        nc.vector.tensor_mul(attnT[p][0:64, q0:q0 + 512],
                                             av_sb[0:64, :], rec)
                    else:
                        # engines can't shift partitions; route via DMA
                        tmp = small.tile([64, 512], BF16, name="tmp",
                                         tag="tmp")
                        nc.vector.tensor_mul(tmp, av_sb[0:64, :], rec)
                        nc.gpsimd.dma_start(
                            out=attnT[p][64:128, q0:q0 + 512], in_=tmp)
                fin.append(finish)
            return fin

        def outproj_qc(qc):
            o_ps = [pspj.tile([128, 512], F32, name=f"o_ps{nt}",
                              tag="pspj") for nt in range(2)]
            for pp in range(NPAIRS):
                for nt in range(2):
                    nc.tensor.matmul(
                        o_ps[nt], attnT[pp][:, qc * 128:(qc + 1) * 128],
                        wo_sb[:, pp, nt * 512:(nt + 1) * 512],
                        start=(pp == 0), stop=(pp == NPAIRS - 1),
                        skip_group_check=True)
            for nt in range(2):
                o_sb = osb.tile([128, 512], F32, name="o_sb", tag="o_sb")
                nc.vector.tensor_copy(o_sb, o_ps[nt])
                eng = nc.sync if (qc + nt) % 2 == 0 else nc.gpsimd
                eng.dma_start(
                    out=out[qc * 128:(qc + 1) * 128,
                            nt * 512:(nt + 1) * 512],
                    in_=o_sb)

        # --- main flow ----------------------------------------------------
        # pair 0 prep: K fully (kt loop needs it all), Q block 0, V tiles
        # 0-2; remaining V tiles and later projections ride the work queues
        for nt in range(4):
            proj_group(0, "k", nt)
        proj_group(0, "q", 0)
        for ktile in range(3):
            v_group(ktile)

        deferred = []
        for p in range(NPAIRS):
            for qt in range(NQT):
                work = list(deferred)
                deferred = []
                if p == 0 and qt == 0:
                    # remaining V tiles, ahead of their consumers (the
                    # attn@V of key tile kt uses the tile issued 3 earlier)
                    work += [lambda kt=k: v_group(kt + 3)
                             for k in range(NKT - 3)]
                    work += [lambda nt=nt2: proj_group(0, "q", nt)
                             for nt2 in range(1, 4)]
                if p + 1 < NPAIRS:
                    if qt == 1:
                        work += [lambda nt=nt2, pn=p + 1:
                                 proj_group(pn, "k", nt)
                                 for nt2 in range(4)]
                        work.append(lambda pn=p + 1: proj_group(pn, "q", 0))
                    elif qt == 2:
                        work += [lambda nt=nt2, pn=p + 1:
                                 proj_group(pn, "q", nt)
                                 for nt2 in range(1, 4)]
                if p == NPAIRS - 1 and qt > 0:
                    work += [lambda qc=qc: outproj_qc(qc)
                             for qc in range((qt - 1) * 4, qt * 4)]
                deferred = attention(p, qt, work)
        for f in deferred:
            f()
        for qc in range(12, 16):
            outproj_qc(qc)

    nc.compile()
    return nc


def _get_compiled():
    global _COMPILED
    if _COMPILED is None:
        _COMPILED = _build()
    return _COMPILED


def make_in_maps(**inputs):
    import ml_dtypes
    bf16 = ml_dtypes.bfloat16
    x = np.asarray(inputs["inputs"], np.float32)
    xTb = [np.ascontiguousarray(x[b].T).astype(bf16) for b in range(B)]
    gslice = {}
    for nm in ("Wq", "Wk", "Wv", "Wo", "bq", "bk"):
        a = np.asarray(inputs[nm], np.float32)
        for g in range(2):
            sl = slice(g * GCOLS, (g + 1) * GCOLS)
            if nm == "Wo":
                gslice[(nm, g)] = np.ascontiguousarray(a[sl, :]).astype(bf16)
            elif nm.startswith("W"):
                gslice[(nm, g)] = np.ascontiguousarray(a[:, sl]).astype(bf16)
            else:
                gslice[(nm, g)] = np.ascontiguousarray(a[sl])
    in_maps = []
    for c in range(NCORES):
        g, b = c // B, c % B
        in_maps.append({
            "xT": xTb[b],
            "wq": gslice[("Wq", g)], "wk": gslice[("Wk", g)],
            "wv": gslice[("Wv", g)], "wo": gslice[("Wo", g)],
            "bq": gslice[("bq", g)], "bk": gslice[("bk", g)],
        })
    return in_maps


def combine(results, bo, bv, Wo):
    out = np.empty((B, S, D), np.float32)
    bo = np.asarray(bo, np.float32)
    bv = np.asarray(bv, np.float32)
    Wo = np.asarray(Wo, np.float32)
    const_row = bo + bv @ Wo
    for b in range(B):
        out[b] = results[b]["out"] + results[B + b]["out"] + const_row
    return out


def kernel(**inputs):
    from concourse import bass_utils
    nc = _get_compiled()
    in_maps = make_in_maps(**inputs)
    res = bass_utils.run_bass_kernel_spmd(
        nc, in_maps, core_ids=list(range(NCORES)))
    return combine(res.results, inputs["bo"], inputs["bv"], inputs["Wo"])
